# revision 31
# baseline (speedup 1.0000x reference)
"""nGPT-style causal attention block on 8 TRN2 NeuronCores.

Sharding: core = (batch b, head-group g); b = core // 4, g = core % 4.
Each core handles 1 batch x 4 heads (512-channel slice) and produces a
partial P = l2norm_cols(Wout)[:, sl] @ oT of shape [DIM, SEQ] in bf16;
the host sums the 4 head-group partials per batch and transposes.

All tensors stay SBUF-resident in bf16 (no DRAM scratch round-trips).
q/k/v projections are weight-stationary; v natural layout comes from
per-head DMA xbar transposes. Weight norms run via ones-column matmul
partition reductions; per-query reciprocal-norm rows are broadcast
across partitions on the idle GpSimd engine (partition_broadcast), so
softmax normalization costs the PE nothing. Causal attention processes
chunk pairs (1024 queries) per key block with a software-pipelined
J-loop (scores/exp of block J emitted before psr/po of J-1); softmax
needs no max pass (scores bounded by qk_scale * sqrt(dh)).
"""
import numpy as np
from contextlib import ExitStack

import concourse.bacc as bacc
import concourse.tile as tile
from concourse import mybir
from concourse.bass_utils import run_bass_kernel_spmd

DIM = 2048          # model dim (= contraction dim of projections)
SEQ = 2048          # sequence length
B = 2               # batch
HEADS = 16
DH = 128            # head dim
NCORES = 8
HPC = 4             # heads per core
ES = HPC * DH       # 512 channels per core
KT = DIM // 128     # 16 contraction tiles
NCH = SEQ // 512    # 4 query chunks of 512
ATT_SCALE = float(DH) ** 0.5

f32 = mybir.dt.float32
f32r = mybir.dt.float32r
bf16 = mybir.dt.bfloat16
AF = mybir.ActivationFunctionType
ALU = mybir.AluOpType


def build_program(repeat=1):
    nc = bacc.Bacc("TRN2", target_bir_lowering=False)

    # ---- per-core DRAM I/O ----
    xT_d = nc.dram_tensor("xT", [128, KT, SEQ], bf16, kind="ExternalInput")
    wqT_d = nc.dram_tensor("wqT", [128, KT, ES], bf16, kind="ExternalInput")
    wkT_d = nc.dram_tensor("wkT", [128, KT, ES], bf16, kind="ExternalInput")
    wvT_d = nc.dram_tensor("wvT", [128, KT, ES], bf16, kind="ExternalInput")
    woT_d = nc.dram_tensor("woT", [128, HPC, DIM], bf16, kind="ExternalInput")
    se_d = nc.dram_tensor("se", [1, ES], f32r, kind="ExternalInput")
    onesr_d = nc.dram_tensor("onesr", [1, 128], f32r, kind="ExternalInput")
    onec_d = nc.dram_tensor("onec", [128, 1], f32r, kind="ExternalInput")
    onecb_d = nc.dram_tensor("onecb", [128, 1], bf16, kind="ExternalInput")
    tri_d = nc.dram_tensor("tri", [128, 128], bf16, kind="ExternalInput")
    out_d = nc.dram_tensor("out", [DIM, SEQ], bf16, kind="ExternalOutput")

    with tile.TileContext(nc) as tc:
      for _rep in range(repeat):
        with ExitStack() as top:
            consts = top.enter_context(tc.tile_pool(name="consts", bufs=1))
            scr = top.enter_context(tc.tile_pool(name="scr", bufs=1,
                                                 space="DRAM"))
            persist = top.enter_context(tc.tile_pool(name="persist", bufs=1))

            # persistent activation stores (bf16)
            v_sb = persist.tile([128, KT, ES], bf16, tag="v")
            q_sb = persist.tile([128, HPC, SEQ], bf16, tag="q")
            k_sb = persist.tile([128, HPC, SEQ], bf16, tag="k")
            oT_sb = persist.tile([128, HPC, SEQ], bf16, tag="oT")

            tri_sb = consts.tile([128, 128], bf16, tag="tri")
            se_sb = consts.tile([1, ES], f32r, tag="se")
            onesr = consts.tile([1, 128], f32r, tag="onesr")
            onec = consts.tile([128, 1], f32r, tag="onec")
            onecb = consts.tile([128, 1], bf16, tag="onecb")
            rn_q = consts.tile([128, HPC], f32, tag="rn_q")
            rn_k = consts.tile([128, HPC], f32, tag="rn_k")
            rnv_row = consts.tile([1, ES], f32, tag="rnv_row")
            comb_row = consts.tile([1, ES], bf16, tag="comb_row")
            sso = consts.tile([128, HPC], f32, tag="sso")
            row_scr = scr.tile([1, ES], f32)
            cmb_scr = scr.tile([128, HPC], f32)

            nc.scalar.dma_start(out=tri_sb, in_=tri_d[:])
            nc.scalar.dma_start(out=se_sb, in_=se_d[:])
            nc.scalar.dma_start(out=onesr, in_=onesr_d[:])
            nc.scalar.dma_start(out=onec, in_=onec_d[:])
            nc.scalar.dma_start(out=onecb, in_=onecb_d[:])

            # ---- weight tiles ----
            wts = top.enter_context(tc.tile_pool(name="wts", bufs=3))
            wq_sb = wts.tile([128, KT, ES], bf16, tag="w")
            wk_sb = wts.tile([128, KT, ES], bf16, tag="w")
            wv_sb = wts.tile([128, KT, ES], bf16, tag="w")

            with ExitStack() as xctx:
                xpool = xctx.enter_context(tc.tile_pool(name="xpool", bufs=1))
                stage = xctx.enter_context(tc.tile_pool(name="stage", bufs=2))
                xt = xpool.tile([128, KT, SEQ], bf16)
                # wv first: the v projection is the first x consumer
                for k in range(KT):
                    nc.sync.dma_start(out=wv_sb[:, k, :], in_=wvT_d[:, k, :])
                    nc.sync.dma_start(out=xt[:, k, :], in_=xT_d[:, k, :])
                nc.scalar.dma_start(out=wq_sb, in_=wqT_d[:])
                nc.scalar.dma_start(out=wk_sb, in_=wkT_d[:])

                # ---- weight row norms from the wT layouts ----
                # ss[ch] = sum_d W[ch,d]^2 via squares + ones-column matmul;
                # rn rows -> partition-major [128,4] via DRAM hop
                def finish_row(ssb, dst_row):
                    nc.scalar.activation(dst_row, ssb, AF.Sqrt)
                    nc.vector.reciprocal_approx_fast(out=dst_row,
                                                     in_=dst_row)

                def row_to_pm(row, pm):
                    nc.scalar.dma_start(out=row_scr, in_=row)
                    for t in range(HPC):
                        nc.scalar.dma_start(
                            out=pm[:, t:t + 1],
                            in_=row_scr[:, t * 128:(t + 1) * 128])

                # ====== phase B: v/q/k projections (weight-stationary) =====
                # The first ~30us are x-DMA-feed-bound, so v for heads 0+1
                # runs with 4 interleaved PSUM accumulators (max PE work per
                # arriving x tile). v for heads 2+3 then runs sequentially
                # with the weight-norm chains interleaved at k granularity.
                with tc.tile_pool(name="brow", bufs=1) as brow, \
                     tc.tile_pool(name="sqp", bufs=1) as sqp:
                    with tc.tile_pool(name="vps", bufs=4,
                                      space="PSUM") as vps:
                        quads = [(0, 0), (0, 1), (1, 0), (1, 1)]
                        tiles = [vps.tile([128, 1024], f32, tag="vq",
                                          name=f"vq{i}")
                                 for i in range(4)]
                        for k in range(KT):
                            for t4, (h, half) in zip(tiles, quads):
                                for j in range(2):
                                    sl = slice(half * 1024 + j * 512,
                                               half * 1024 + (j + 1) * 512)
                                    nc.tensor.matmul(
                                        t4[:, j * 512:(j + 1) * 512],
                                        wv_sb[:, k, h * 128:(h + 1) * 128],
                                        xt[:, k, sl],
                                        start=(k == 0), stop=(k == KT - 1))
                        for vh in (0, 1):
                            vst = stage.tile([128, SEQ], bf16, tag="vst")
                            for t4, (h, half) in zip(tiles, quads):
                                if h != vh:
                                    continue
                                hs = slice(half * 1024, (half + 1) * 1024)
                                nc.vector.tensor_copy(vst[:, hs], t4)
                            # v natural layout via xbar transpose
                            nc.sync.dma_start(
                                out=v_sb[:, :, vh * 128:(vh + 1) * 128],
                                in_=vst, transpose=True)

                    # phase C's score PSUM pool: pre-opened here (2 banks)
                    # so C's first scores can issue during B's tail
                    cpsc = top.enter_context(
                        tc.tile_pool(name="cpsc", bufs=2, space="PSUM"))

                    with tc.tile_pool(name="qkps", bufs=2,
                                      space="PSUM") as qkps, \
                         tc.tile_pool(name="nrm_ps", bufs=1,
                                      space="PSUM") as nrm_ps:

                        def proj_half(w_sb, h, half, wn_sb=None, ssb=None):
                            """One projection half; optionally interleave a
                            weight-norm square+reduce chain per k tile."""
                            qps = qkps.tile([128, 1024], f32, tag="qps",
                                            name=f"qps_{h}_{half}")
                            for k in range(KT):
                                if wn_sb is not None:
                                    sq = sqp.tile([128, ES], f32r, tag="sq",
                                                  name=f"wsq{k}")
                                    nc.vector.tensor_mul(
                                        sq, wn_sb[:, k, :], wn_sb[:, k, :])
                                    nc.tensor.matmul(ssb, onec, sq,
                                                     start=(k == 0),
                                                     stop=(k == KT - 1))
                                for j in range(2):
                                    sl = slice(half * 1024 + j * 512,
                                               half * 1024 + (j + 1) * 512)
                                    nc.tensor.matmul(
                                        qps[:, j * 512:(j + 1) * 512],
                                        w_sb[:, k, h * 128:(h + 1) * 128],
                                        xt[:, k, sl],
                                        start=(k == 0), stop=(k == KT - 1))
                            return qps

                        # v heads 2,3 with wq/wk/wv norm chains interleaved
                        rq_row = brow.tile([1, ES], f32, tag="wrow")
                        rk_row = brow.tile([1, ES], f32, tag="rkrow")
                        for vh in (2, 3):
                            vst = stage.tile([128, SEQ], bf16, tag="vst")
                            for half in range(2):
                                wn, row, rn = (
                                    (wq_sb, rq_row, rn_q),
                                    (wk_sb, rk_row, rn_k),
                                    (wv_sb, rnv_row, None),
                                    (None, None, None))[(vh - 2) * 2 + half]
                                ssb = None
                                if wn is not None:
                                    ssb = nrm_ps.tile([1, ES], f32, tag="ss",
                                                      name=f"ssn{vh}{half}")
                                qps = proj_half(wv_sb, vh, half, wn, ssb)
                                nc.vector.tensor_copy(
                                    vst[:, half * 1024:(half + 1) * 1024],
                                    qps)
                                if wn is not None:
                                    finish_row(ssb, row)
                                    if rn is not None:
                                        row_to_pm(row, rn)
                            nc.sync.dma_start(
                                out=v_sb[:, :, vh * 128:(vh + 1) * 128],
                                in_=vst, transpose=True)

                        # wout col norms (strip loads) -> comb_pm (phase C)
                        for t in range(HPC):
                            ws = stage.tile([128, DIM], bf16, tag="vst",
                                            name=f"wos{t}")
                            nc.scalar.dma_start(out=ws, in_=woT_d[:, t, :])
                            nc.scalar.activation(ws, ws, AF.Square,
                                                 accum_out=sso[:, t:t + 1])
                        nc.scalar.activation(sso, sso, AF.Sqrt)
                        nc.vector.reciprocal_approx_fast(out=sso, in_=sso)
                        nc.scalar.dma_start(out=cmb_scr, in_=sso[:])
                        wo_row = brow.tile([1, ES], f32, tag="wrow")
                        for t in range(HPC):
                            nc.scalar.dma_start(
                                out=wo_row[:, t * 128:(t + 1) * 128],
                                in_=cmb_scr[:, t:t + 1])
                        nc.vector.tensor_mul(comb_row, wo_row, rnv_row)

                        def qknorm(dst, h, is_q):
                            # per-query l2 norm; reciprocal row broadcast on
                            # GpSimd; qk scale folded as per-partition scalar
                            for c in range(NCH):
                                cs = slice(c * 512, (c + 1) * 512)
                                sq = sqp.tile([128, 512], f32r, tag="sq")
                                nc.vector.tensor_mul(
                                    sq, dst[:, h, cs], dst[:, h, cs])
                                ssb = nrm_ps.tile([1, 512], f32, tag="ss")
                                nc.tensor.matmul(ssb, onec, sq,
                                                 start=True, stop=True)
                                srow = brow.tile([1, 512], f32, tag="srow")
                                nc.scalar.activation(srow, ssb, AF.Sqrt)
                                r2 = brow.tile([1, 512], f32, tag="r2")
                                nc.vector.reciprocal_approx_fast(
                                    out=r2, in_=srow)
                                rrow = brow.tile([1, 512], f32r, tag="rrow")
                                nc.vector.tensor_copy(rrow, r2)
                                bc = nrm_ps.tile([128, 512], f32, tag="bc")
                                lhs = (se_sb[:, h * 128:(h + 1) * 128]
                                       if is_q else onesr[:, :])
                                nc.tensor.matmul(bc, lhs, rrow,
                                                 start=True, stop=True)
                                nc.vector.tensor_mul(
                                    dst[:, h, cs], dst[:, h, cs], bc)

                        for h in range(HPC):
                            for half in range(2):
                                qps = proj_half(wq_sb, h, half)
                                nc.vector.tensor_scalar_mul(
                                    q_sb[:, h,
                                         half * 1024:(half + 1) * 1024],
                                    qps, rn_q[:, h:h + 1])
                            qknorm(q_sb, h, True)

                        for h in range(HPC):
                            for half in range(2):
                                qps = proj_half(wk_sb, h, half)
                                nc.vector.tensor_scalar_mul(
                                    k_sb[:, h,
                                         half * 1024:(half + 1) * 1024],
                                    qps, rn_k[:, h:h + 1])
                            qknorm(k_sb, h, False)

            # ---- wo resident for phase D (loads while C runs) ----
            wop = top.enter_context(tc.tile_pool(name="wop", bufs=1))
            wo_sb = wop.tile([128, HPC, DIM], bf16, tag="wo")
            nc.sync.dma_start(out=wo_sb, in_=woT_d[:])

            # ================= phase C: causal attention ===================
            # chunk pairs: one kT/vT weight load serves both 512-q chunks.
            # J-loop is software-pipelined: scores/exp for key block J are
            # emitted before the psr/po matmuls of block J-1, so the PE
            # never waits on the ACT exp. Evictions are DVE/GpSimd-only.
            with tc.tile_pool(name="po_ps", bufs=3, space="PSUM") as po_ps, \
                 tc.tile_pool(name="psr_ps", bufs=3, space="PSUM") as psr_ps, \
                 tc.tile_pool(name="epool", bufs=4) as epool, \
                 tc.tile_pool(name="crow", bufs=2) as crow_pool, \
                 tc.tile_pool(name="cbs", bufs=2) as cbs_pool:

                pend = []

                def evict1(h, c, po, psr):
                    r2 = crow_pool.tile([1, 512], f32, tag="cr2")
                    nc.vector.reciprocal_approx_fast(out=r2, in_=psr)
                    rrow = crow_pool.tile([1, 512], bf16, tag="crow")
                    nc.vector.tensor_copy(rrow, r2)

                    def evict2():
                        bc = cpsc.tile([128, 512], f32, tag="psc")
                        nc.tensor.matmul(
                            bc, comb_row[:, h * 128:(h + 1) * 128],
                            rrow, start=True, stop=True)
                        bcs = cbs_pool.tile([128, 512], f32, tag="cbcs")
                        nc.vector.tensor_copy(bcs, bc)
                        nc.vector.tensor_mul(
                            oT_sb[:, h, c * 512:(c + 1) * 512], po, bcs)
                    return evict2

                for h in range(HPC):
                    ksl = lambda J: k_sb[:, h, J * 128:(J + 1) * 128]
                    vsl = lambda J: v_sb[:, J, h * 128:(h + 1) * 128]
                    for cp in range(2):
                        c0, c1 = 2 * cp, 2 * cp + 1
                        q0 = c0 * 512
                        po0 = po_ps.tile([128, 512], f32, tag="po")
                        po1 = po_ps.tile([128, 512], f32, tag="po")
                        ps0 = psr_ps.tile([1, 512], f32, tag="psr")
                        ps1 = psr_ps.tile([1, 512], f32, tag="psr")
                        nj = 4 * c1 + 4
                        prev = None
                        for J in range(nj + 1):
                            if J < nj:
                                m0 = J - 4 * c0   # diag block idx in c0
                                m1 = J - 4 * c1
                                lo = (0 if m0 < 0 else
                                      (512 if m0 > 3 else m0 * 128))
                                h1 = max(m1, 0) * 128
                                e0 = None
                                if lo < 512:
                                    p0 = cpsc.tile([128, 512], f32,
                                                   tag="psc")
                                    nc.tensor.matmul(
                                        p0[:, lo:], ksl(J),
                                        q_sb[:, h, q0 + lo:q0 + 512],
                                        start=True, stop=True)
                                    e0 = epool.tile([128, 512], bf16,
                                                    tag="esb")
                                    nc.scalar.activation(
                                        e0[:, lo:], p0[:, lo:], AF.Exp,
                                        scale=ATT_SCALE)
                                    if m0 >= 0:
                                        nc.vector.tensor_mul(
                                            e0[:, lo:lo + 128],
                                            e0[:, lo:lo + 128], tri_sb)
                                p1 = cpsc.tile([128, 512], f32, tag="psc")
                                nc.tensor.matmul(
                                    p1[:, h1:], ksl(J),
                                    q_sb[:, h, q0 + 512 + h1:q0 + 1024],
                                    start=True, stop=True)
                                e1 = epool.tile([128, 512], bf16, tag="esb")
                                nc.scalar.activation(e1[:, h1:], p1[:, h1:],
                                                     AF.Exp, scale=ATT_SCALE)
                                if m1 >= 0:
                                    nc.vector.tensor_mul(
                                        e1[:, h1:h1 + 128],
                                        e1[:, h1:h1 + 128], tri_sb)
                            if prev is not None:
                                Jp, f0, f1, lp, hp = prev
                                if f0 is not None:
                                    nc.tensor.matmul(
                                        ps0[:, lp:], onecb, f0[:, lp:],
                                        start=(Jp == 0),
                                        stop=(Jp == 4 * c0 + 3))
                                nc.tensor.matmul(
                                    ps1[:, hp:], onecb, f1[:, hp:],
                                    start=(Jp == 0), stop=(Jp == nj - 1))
                                if f0 is not None:
                                    nc.tensor.matmul(
                                        po0[:, lp:], vsl(Jp), f0[:, lp:],
                                        start=(Jp == 0),
                                        stop=(Jp == 4 * c0 + 3))
                                nc.tensor.matmul(
                                    po1[:, hp:], vsl(Jp), f1[:, hp:],
                                    start=(Jp == 0), stop=(Jp == nj - 1))
                            prev = ((J, e0, e1, lo, h1) if J < nj else None)
                            if J == 1 and pend:
                                for f in pend:
                                    f()
                                pend = []
                            if J == 4 * c0 + 4:
                                ev_c0 = evict1(h, c0, po0, ps0)
                            elif J == 4 * c0 + 6:
                                ev_c0()
                            elif J == nj:
                                pend.append(evict1(h, c1, po1, ps1))
                for f in pend:
                    f()

            # ================= phase D: output projection ==================
            with tc.tile_pool(name="d_ps", bufs=3, space="PSUM") as d_ps, \
                 tc.tile_pool(name="opool", bufs=3) as opool:
                for d in range(DIM // 128):
                    for half in range(2):
                        dps = d_ps.tile([128, 1024], f32, tag="dps")
                        for t in range(HPC):
                            for j in range(2):
                                js = slice(half * 1024 + j * 512,
                                           half * 1024 + (j + 1) * 512)
                                nc.tensor.matmul(
                                    dps[:, j * 512:(j + 1) * 512],
                                    wo_sb[:, t, d * 128:(d + 1) * 128],
                                    oT_sb[:, t, js],
                                    start=(t == 0), stop=(t == HPC - 1))
                        ob = opool.tile([128, 1024], bf16, tag="ob")
                        if half == 0:
                            nc.vector.tensor_copy(ob, dps)
                        else:
                            nc.scalar.copy(ob, dps)
                        nc.sync.dma_start(
                            out=out_d[d * 128:(d + 1) * 128,
                                      half * 1024:(half + 1) * 1024],
                            in_=ob[:])

    nc.compile()
    return nc


_CACHE = {}


def _get_program(repeat=1):
    if repeat not in _CACHE:
        _CACHE[repeat] = build_program(repeat)
    return _CACHE[repeat]


def _make_in_maps(x, Wq, Wk, Wv, Wout, qk_scale):
    nbf = mybir.dt.np(bf16)
    tri = np.triu(np.ones((128, 128), dtype=np.float32)).astype(nbf)
    onec = np.ones((128, 1), dtype=np.float32)
    onecb = np.ones((128, 1), dtype=np.float32).astype(nbf)

    def t3(a, kt, p, n):  # [kt*p, n] -> [p, kt, n]
        return np.ascontiguousarray(
            a.reshape(kt, p, n).transpose(1, 0, 2)).astype(nbf)

    in_maps = []
    for core in range(NCORES):
        b, g = divmod(core, HPC)
        sl = slice(g * ES, (g + 1) * ES)
        in_maps.append({
            "xT": t3(x[b].T, KT, 128, SEQ),
            "wqT": t3(Wq[sl].T, KT, 128, ES),
            "wkT": t3(Wk[sl].T, KT, 128, ES),
            "wvT": t3(Wv[sl].T, KT, 128, ES),
            "woT": t3(Wout[:, sl].T, HPC, 128, DIM),
            "se": np.ascontiguousarray(
                (qk_scale[sl] * DIM).reshape(1, ES)).astype(np.float32),
            "onesr": np.ones((1, 128), dtype=np.float32),
            "onec": onec,
            "onecb": onecb,
            "tri": tri,
        })
    return in_maps


def _assemble(results):
    out = np.empty((B, SEQ, DIM), dtype=np.float32)
    for b in range(B):
        acc = results[4 * b]["out"].astype(np.float32)
        for g in range(1, HPC):
            acc = acc + results[4 * b + g]["out"].astype(np.float32)
        out[b] = acc.T
    return out


def kernel(x, Wq, Wk, Wv, Wout, qk_scale):
    nc = _get_program()
    in_maps = _make_in_maps(x, Wq, Wk, Wv, Wout, qk_scale)
    res = run_bass_kernel_spmd(nc, in_maps, core_ids=list(range(NCORES)))
    return _assemble(res.results)


# revision 33
# speedup vs baseline: 3.0747x; 3.0747x over previous
"""nGPT-style causal attention block on 8 TRN2 NeuronCores.

Sharding: core = (batch b, head-group g); b = core // 4, g = core % 4.
Each core handles 1 batch x 4 heads (512-channel slice) and produces a
partial P = l2norm_cols(Wout)[:, sl] @ oT of shape [DIM, SEQ] in bf16;
the host sums the 4 head-group partials per batch and transposes.

All tensors stay SBUF-resident in bf16 (no DRAM scratch round-trips).
q/k/v projections are weight-stationary; v natural layout comes from
per-head DMA xbar transposes. Weight norms run via ones-column matmul
partition reductions; per-query reciprocal norms use fast approximate
reciprocals and rank-1 matmul broadcasts into PSUM. Causal attention
processes chunk pairs (1024 queries) per key block with a software-
pipelined J-loop (scores/exp of block J emitted before the psr/po
matmuls of block J-1, so the PE never waits on ACT) and deferred
two-stage evictions; softmax needs no max pass (scores are bounded
by qk_scale * sqrt(dh)).
"""
import numpy as np
from contextlib import ExitStack

import concourse.bacc as bacc
import concourse.tile as tile
from concourse import mybir
from concourse.bass_utils import run_bass_kernel_spmd

DIM = 2048          # model dim (= contraction dim of projections)
SEQ = 2048          # sequence length
B = 2               # batch
HEADS = 16
DH = 128            # head dim
NCORES = 8
HPC = 4             # heads per core
ES = HPC * DH       # 512 channels per core
KT = DIM // 128     # 16 contraction tiles
NCH = SEQ // 512    # 4 query chunks of 512
ATT_SCALE = float(DH) ** 0.5

f32 = mybir.dt.float32
f32r = mybir.dt.float32r
bf16 = mybir.dt.bfloat16
AF = mybir.ActivationFunctionType
ALU = mybir.AluOpType


def build_program(repeat=1):
    nc = bacc.Bacc("TRN2", target_bir_lowering=False)

    # ---- per-core DRAM I/O ----
    xT_d = nc.dram_tensor("xT", [128, KT, SEQ], bf16, kind="ExternalInput")
    wqT_d = nc.dram_tensor("wqT", [128, KT, ES], bf16, kind="ExternalInput")
    wkT_d = nc.dram_tensor("wkT", [128, KT, ES], bf16, kind="ExternalInput")
    wvT_d = nc.dram_tensor("wvT", [128, KT, ES], bf16, kind="ExternalInput")
    woT_d = nc.dram_tensor("woT", [128, HPC, DIM], bf16, kind="ExternalInput")
    se_d = nc.dram_tensor("se", [1, ES], f32r, kind="ExternalInput")
    onesr_d = nc.dram_tensor("onesr", [1, 128], f32r, kind="ExternalInput")
    onec_d = nc.dram_tensor("onec", [128, 1], f32r, kind="ExternalInput")
    onecb_d = nc.dram_tensor("onecb", [128, 1], bf16, kind="ExternalInput")
    tri_d = nc.dram_tensor("tri", [128, 128], bf16, kind="ExternalInput")
    out_d = nc.dram_tensor("out", [DIM, SEQ], bf16, kind="ExternalOutput")

    with tile.TileContext(nc) as tc:
      for _rep in range(repeat):
        with ExitStack() as top:
            consts = top.enter_context(tc.tile_pool(name="consts", bufs=1))
            scr = top.enter_context(tc.tile_pool(name="scr", bufs=1,
                                                 space="DRAM"))
            persist = top.enter_context(tc.tile_pool(name="persist", bufs=1))

            # persistent activation stores (bf16)
            v_sb = persist.tile([128, KT, ES], bf16, tag="v")
            q_sb = persist.tile([128, HPC, SEQ], bf16, tag="q")
            k_sb = persist.tile([128, HPC, SEQ], bf16, tag="k")
            oT_sb = persist.tile([128, HPC, SEQ], bf16, tag="oT")

            tri_sb = consts.tile([128, 128], bf16, tag="tri")
            se_sb = consts.tile([1, ES], f32r, tag="se")
            onesr = consts.tile([1, 128], f32r, tag="onesr")
            onec = consts.tile([128, 1], f32r, tag="onec")
            onecb = consts.tile([128, 1], bf16, tag="onecb")
            rn_q = consts.tile([128, HPC], f32, tag="rn_q")
            rn_k = consts.tile([128, HPC], f32, tag="rn_k")
            rnv_row = consts.tile([1, ES], f32, tag="rnv_row")
            comb_row = consts.tile([1, ES], bf16, tag="comb_row")
            sso = consts.tile([128, HPC], f32, tag="sso")
            row_scr = scr.tile([1, ES], f32)
            cmb_scr = scr.tile([128, HPC], f32)

            nc.scalar.dma_start(out=tri_sb, in_=tri_d[:])
            nc.scalar.dma_start(out=se_sb, in_=se_d[:])
            nc.scalar.dma_start(out=onesr, in_=onesr_d[:])
            nc.scalar.dma_start(out=onec, in_=onec_d[:])
            nc.scalar.dma_start(out=onecb, in_=onecb_d[:])

            # ---- weight tiles ----
            wts = top.enter_context(tc.tile_pool(name="wts", bufs=3))
            wq_sb = wts.tile([128, KT, ES], bf16, tag="w")
            wk_sb = wts.tile([128, KT, ES], bf16, tag="w")
            wv_sb = wts.tile([128, KT, ES], bf16, tag="w")

            with ExitStack() as xctx:
                xpool = xctx.enter_context(tc.tile_pool(name="xpool", bufs=1))
                stage = xctx.enter_context(tc.tile_pool(name="stage", bufs=2))
                xt = xpool.tile([128, KT, SEQ], bf16)
                # wv first: the v projection is the first x consumer.
                # First k-tiles split in 512-col pieces so MM k=0 starts
                # as soon as ~256KB has landed.
                for k in range(KT):
                    nc.sync.dma_start(out=wv_sb[:, k, :], in_=wvT_d[:, k, :])
                    if k < 2:
                        for j4 in range(4):
                            js = slice(j4 * 512, (j4 + 1) * 512)
                            nc.sync.dma_start(out=xt[:, k, js],
                                              in_=xT_d[:, k, js])
                    else:
                        nc.sync.dma_start(out=xt[:, k, :], in_=xT_d[:, k, :])
                nc.scalar.dma_start(out=wq_sb, in_=wqT_d[:])
                nc.scalar.dma_start(out=wk_sb, in_=wkT_d[:])

                # ---- weight row norms from the wT layouts ----
                # ss[ch] = sum_d W[ch,d]^2 via squares + ones-column matmul;
                # rn rows -> partition-major [128,4] via DRAM hop
                def finish_row(ssb, dst_row):
                    nc.scalar.activation(dst_row, ssb, AF.Sqrt)
                    nc.vector.reciprocal_approx_fast(out=dst_row,
                                                     in_=dst_row)

                def row_to_pm(row, pm):
                    nc.scalar.dma_start(out=row_scr, in_=row)
                    for t in range(HPC):
                        nc.scalar.dma_start(
                            out=pm[:, t:t + 1],
                            in_=row_scr[:, t * 128:(t + 1) * 128])

                # ====== phase B: v/q/k projections (weight-stationary) =====
                # The first ~30us are x-DMA-feed-bound, so v for heads 0+1
                # runs with 4 interleaved PSUM accumulators (max PE work per
                # arriving x tile). v for heads 2+3 then runs sequentially
                # with the weight-norm chains interleaved at k granularity.
                with tc.tile_pool(name="brow", bufs=1) as brow, \
                     tc.tile_pool(name="sqp", bufs=1) as sqp:
                    with tc.tile_pool(name="vps", bufs=4,
                                      space="PSUM") as vps:
                        quads = [(0, 0), (0, 1), (1, 0), (1, 1)]
                        tiles = [vps.tile([128, 1024], f32, tag="vq",
                                          name=f"vq{i}")
                                 for i in range(4)]
                        for k in range(KT):
                            for t4, (h, half) in zip(tiles, quads):
                                for j in range(2):
                                    sl = slice(half * 1024 + j * 512,
                                               half * 1024 + (j + 1) * 512)
                                    nc.tensor.matmul(
                                        t4[:, j * 512:(j + 1) * 512],
                                        wv_sb[:, k, h * 128:(h + 1) * 128],
                                        xt[:, k, sl],
                                        start=(k == 0), stop=(k == KT - 1))
                        for vh in (0, 1):
                            vst = stage.tile([128, SEQ], bf16, tag="vst")
                            for t4, (h, half) in zip(tiles, quads):
                                if h != vh:
                                    continue
                                hs = slice(half * 1024, (half + 1) * 1024)
                                nc.vector.tensor_copy(vst[:, hs], t4)
                            # v natural layout via xbar transpose
                            nc.sync.dma_start(
                                out=v_sb[:, :, vh * 128:(vh + 1) * 128],
                                in_=vst, transpose=True)

                    # phase C's score PSUM pool: pre-opened here (2 banks)
                    # so C's first scores can issue during B's tail
                    cpsc = top.enter_context(
                        tc.tile_pool(name="cpsc", bufs=2, space="PSUM"))

                    with tc.tile_pool(name="qkps", bufs=2,
                                      space="PSUM") as qkps, \
                         tc.tile_pool(name="nrm_ps", bufs=1,
                                      space="PSUM") as nrm_ps:

                        def proj_half(w_sb, h, half, wn_sb=None, ssb=None):
                            """One projection half; optionally interleave a
                            weight-norm square+reduce chain per k tile."""
                            qps = qkps.tile([128, 1024], f32, tag="qps",
                                            name=f"qps_{h}_{half}")
                            for k in range(KT):
                                if wn_sb is not None:
                                    sq = sqp.tile([128, ES], f32r, tag="sq",
                                                  name=f"wsq{k}")
                                    nc.vector.tensor_mul(
                                        sq, wn_sb[:, k, :], wn_sb[:, k, :])
                                    nc.tensor.matmul(ssb, onec, sq,
                                                     start=(k == 0),
                                                     stop=(k == KT - 1))
                                for j in range(2):
                                    sl = slice(half * 1024 + j * 512,
                                               half * 1024 + (j + 1) * 512)
                                    nc.tensor.matmul(
                                        qps[:, j * 512:(j + 1) * 512],
                                        w_sb[:, k, h * 128:(h + 1) * 128],
                                        xt[:, k, sl],
                                        start=(k == 0), stop=(k == KT - 1))
                            return qps

                        # v heads 2,3 with wq/wk/wv norm chains interleaved
                        rq_row = brow.tile([1, ES], f32, tag="wrow")
                        rk_row = brow.tile([1, ES], f32, tag="rkrow")
                        for vh in (2, 3):
                            vst = stage.tile([128, SEQ], bf16, tag="vst")
                            for half in range(2):
                                wn, row, rn = (
                                    (wq_sb, rq_row, rn_q),
                                    (wk_sb, rk_row, rn_k),
                                    (wv_sb, rnv_row, None),
                                    (None, None, None))[(vh - 2) * 2 + half]
                                ssb = None
                                if wn is not None:
                                    ssb = nrm_ps.tile([1, ES], f32, tag="ss",
                                                      name=f"ssn{vh}{half}")
                                qps = proj_half(wv_sb, vh, half, wn, ssb)
                                nc.vector.tensor_copy(
                                    vst[:, half * 1024:(half + 1) * 1024],
                                    qps)
                                if wn is not None:
                                    finish_row(ssb, row)
                                    if rn is not None:
                                        row_to_pm(row, rn)
                            nc.sync.dma_start(
                                out=v_sb[:, :, vh * 128:(vh + 1) * 128],
                                in_=vst, transpose=True)

                        # wout col norms (strip loads) -> comb_pm (phase C)
                        for t in range(HPC):
                            ws = stage.tile([128, DIM], bf16, tag="vst",
                                            name=f"wos{t}")
                            nc.scalar.dma_start(out=ws, in_=woT_d[:, t, :])
                            nc.scalar.activation(ws, ws, AF.Square,
                                                 accum_out=sso[:, t:t + 1])
                        nc.scalar.activation(sso, sso, AF.Sqrt)
                        nc.vector.reciprocal_approx_fast(out=sso, in_=sso)
                        nc.scalar.dma_start(out=cmb_scr, in_=sso[:])
                        wo_row = brow.tile([1, ES], f32, tag="wrow")
                        for t in range(HPC):
                            nc.scalar.dma_start(
                                out=wo_row[:, t * 128:(t + 1) * 128],
                                in_=cmb_scr[:, t:t + 1])
                        nc.vector.tensor_mul(comb_row, wo_row, rnv_row)

                        def qknorm(dst, h, is_q):
                            # per-query l2 norm; reciprocal row broadcast on
                            # GpSimd; qk scale folded as per-partition scalar
                            for c in range(NCH):
                                cs = slice(c * 512, (c + 1) * 512)
                                sq = sqp.tile([128, 512], f32r, tag="sq")
                                nc.vector.tensor_mul(
                                    sq, dst[:, h, cs], dst[:, h, cs])
                                ssb = nrm_ps.tile([1, 512], f32, tag="ss")
                                nc.tensor.matmul(ssb, onec, sq,
                                                 start=True, stop=True)
                                srow = brow.tile([1, 512], f32, tag="srow")
                                nc.scalar.activation(srow, ssb, AF.Sqrt)
                                r2 = brow.tile([1, 512], f32, tag="r2")
                                nc.vector.reciprocal_approx_fast(
                                    out=r2, in_=srow)
                                rrow = brow.tile([1, 512], f32r, tag="rrow")
                                nc.vector.tensor_copy(rrow, r2)
                                bc = nrm_ps.tile([128, 512], f32, tag="bc")
                                lhs = (se_sb[:, h * 128:(h + 1) * 128]
                                       if is_q else onesr[:, :])
                                nc.tensor.matmul(bc, lhs, rrow,
                                                 start=True, stop=True)
                                nc.vector.tensor_mul(
                                    dst[:, h, cs], dst[:, h, cs], bc)

                        for h in range(HPC):
                            for half in range(2):
                                qps = proj_half(wq_sb, h, half)
                                nc.vector.tensor_scalar_mul(
                                    q_sb[:, h,
                                         half * 1024:(half + 1) * 1024],
                                    qps, rn_q[:, h:h + 1])
                            qknorm(q_sb, h, True)

                        for h in range(HPC):
                            for half in range(2):
                                qps = proj_half(wk_sb, h, half)
                                nc.vector.tensor_scalar_mul(
                                    k_sb[:, h,
                                         half * 1024:(half + 1) * 1024],
                                    qps, rn_k[:, h:h + 1])
                            qknorm(k_sb, h, False)

            # ---- wo resident for phase D (loads while C runs) ----
            wop = top.enter_context(tc.tile_pool(name="wop", bufs=1))
            wo_sb = wop.tile([128, HPC, DIM], bf16, tag="wo")
            nc.sync.dma_start(out=wo_sb, in_=woT_d[:])

            # ================= phase C: causal attention ===================
            # chunk pairs: one kT/vT weight load serves both 512-q chunks.
            # J-loop is software-pipelined: scores/exp for key block J are
            # emitted before the psr/po matmuls of block J-1, so the PE
            # never waits on the ACT exp. Evictions are DVE/GpSimd-only.
            with tc.tile_pool(name="po_ps", bufs=3, space="PSUM") as po_ps, \
                 tc.tile_pool(name="psr_ps", bufs=3, space="PSUM") as psr_ps, \
                 tc.tile_pool(name="epool", bufs=4) as epool, \
                 tc.tile_pool(name="crow", bufs=2) as crow_pool, \
                 tc.tile_pool(name="cbs", bufs=2) as cbs_pool:

                pend = []

                def evict1(h, c, po, psr):
                    r2 = crow_pool.tile([1, 512], f32, tag="cr2")
                    nc.vector.reciprocal_approx_fast(out=r2, in_=psr)
                    rrow = crow_pool.tile([1, 512], bf16, tag="crow")
                    nc.vector.tensor_copy(rrow, r2)

                    def evict2():
                        bc = cpsc.tile([128, 512], f32, tag="psc")
                        nc.tensor.matmul(
                            bc, comb_row[:, h * 128:(h + 1) * 128],
                            rrow, start=True, stop=True)
                        bcs = cbs_pool.tile([128, 512], f32, tag="cbcs")
                        nc.vector.tensor_copy(bcs, bc)
                        nc.vector.tensor_mul(
                            oT_sb[:, h, c * 512:(c + 1) * 512], po, bcs)
                    return evict2

                for h in range(HPC):
                    ksl = lambda J: k_sb[:, h, J * 128:(J + 1) * 128]
                    vsl = lambda J: v_sb[:, J, h * 128:(h + 1) * 128]
                    for cp in range(2):
                        c0, c1 = 2 * cp, 2 * cp + 1
                        q0 = c0 * 512
                        po0 = po_ps.tile([128, 512], f32, tag="po")
                        po1 = po_ps.tile([128, 512], f32, tag="po")
                        ps0 = psr_ps.tile([1, 512], f32, tag="psr")
                        ps1 = psr_ps.tile([1, 512], f32, tag="psr")
                        nj = 4 * c1 + 4
                        prev = None
                        for J in range(nj + 1):
                            if J < nj:
                                m0 = J - 4 * c0   # diag block idx in c0
                                m1 = J - 4 * c1
                                lo = (0 if m0 < 0 else
                                      (512 if m0 > 3 else m0 * 128))
                                h1 = max(m1, 0) * 128
                                e0 = None
                                if lo < 512:
                                    p0 = cpsc.tile([128, 512], f32,
                                                   tag="psc")
                                    nc.tensor.matmul(
                                        p0[:, lo:], ksl(J),
                                        q_sb[:, h, q0 + lo:q0 + 512],
                                        start=True, stop=True)
                                    e0 = epool.tile([128, 512], bf16,
                                                    tag="esb")
                                    nc.scalar.activation(
                                        e0[:, lo:], p0[:, lo:], AF.Exp,
                                        scale=ATT_SCALE)
                                    if m0 >= 0:
                                        nc.vector.tensor_mul(
                                            e0[:, lo:lo + 128],
                                            e0[:, lo:lo + 128], tri_sb)
                                p1 = cpsc.tile([128, 512], f32, tag="psc")
                                nc.tensor.matmul(
                                    p1[:, h1:], ksl(J),
                                    q_sb[:, h, q0 + 512 + h1:q0 + 1024],
                                    start=True, stop=True)
                                e1 = epool.tile([128, 512], bf16, tag="esb")
                                nc.scalar.activation(e1[:, h1:], p1[:, h1:],
                                                     AF.Exp, scale=ATT_SCALE)
                                if m1 >= 0:
                                    nc.vector.tensor_mul(
                                        e1[:, h1:h1 + 128],
                                        e1[:, h1:h1 + 128], tri_sb)
                            if prev is not None:
                                Jp, f0, f1, lp, hp = prev
                                if f0 is not None:
                                    nc.tensor.matmul(
                                        ps0[:, lp:], onecb, f0[:, lp:],
                                        start=(Jp == 0),
                                        stop=(Jp == 4 * c0 + 3))
                                nc.tensor.matmul(
                                    ps1[:, hp:], onecb, f1[:, hp:],
                                    start=(Jp == 0), stop=(Jp == nj - 1))
                                if f0 is not None:
                                    nc.tensor.matmul(
                                        po0[:, lp:], vsl(Jp), f0[:, lp:],
                                        start=(Jp == 0),
                                        stop=(Jp == 4 * c0 + 3))
                                nc.tensor.matmul(
                                    po1[:, hp:], vsl(Jp), f1[:, hp:],
                                    start=(Jp == 0), stop=(Jp == nj - 1))
                            prev = ((J, e0, e1, lo, h1) if J < nj else None)
                            if J == 1 and pend:
                                for f in pend:
                                    f()
                                pend = []
                            if J == 4 * c0 + 4:
                                ev_c0 = evict1(h, c0, po0, ps0)
                            elif J == 4 * c0 + 6:
                                ev_c0()
                            elif J == nj:
                                pend.append(evict1(h, c1, po1, ps1))
                for f in pend:
                    f()

            # ================= phase D: output projection ==================
            with tc.tile_pool(name="d_ps", bufs=3, space="PSUM") as d_ps, \
                 tc.tile_pool(name="opool", bufs=3) as opool:
                for d in range(DIM // 128):
                    for half in range(2):
                        dps = d_ps.tile([128, 1024], f32, tag="dps")
                        for t in range(HPC):
                            for j in range(2):
                                js = slice(half * 1024 + j * 512,
                                           half * 1024 + (j + 1) * 512)
                                nc.tensor.matmul(
                                    dps[:, j * 512:(j + 1) * 512],
                                    wo_sb[:, t, d * 128:(d + 1) * 128],
                                    oT_sb[:, t, js],
                                    start=(t == 0), stop=(t == HPC - 1))
                        ob = opool.tile([128, 1024], bf16, tag="ob")
                        if half == 0:
                            nc.vector.tensor_copy(ob, dps)
                        else:
                            nc.scalar.copy(ob, dps)
                        nc.sync.dma_start(
                            out=out_d[d * 128:(d + 1) * 128,
                                      half * 1024:(half + 1) * 1024],
                            in_=ob[:])

    nc.compile()
    return nc


_CACHE = {}


def _get_program(repeat=1):
    if repeat not in _CACHE:
        _CACHE[repeat] = build_program(repeat)
    return _CACHE[repeat]


def _make_in_maps(x, Wq, Wk, Wv, Wout, qk_scale):
    nbf = mybir.dt.np(bf16)
    tri = np.triu(np.ones((128, 128), dtype=np.float32)).astype(nbf)
    onec = np.ones((128, 1), dtype=np.float32)
    onecb = np.ones((128, 1), dtype=np.float32).astype(nbf)

    def t3(a, kt, p, n):  # [kt*p, n] -> [p, kt, n]
        return np.ascontiguousarray(
            a.reshape(kt, p, n).transpose(1, 0, 2)).astype(nbf)

    in_maps = []
    for core in range(NCORES):
        b, g = divmod(core, HPC)
        sl = slice(g * ES, (g + 1) * ES)
        in_maps.append({
            "xT": t3(x[b].T, KT, 128, SEQ),
            "wqT": t3(Wq[sl].T, KT, 128, ES),
            "wkT": t3(Wk[sl].T, KT, 128, ES),
            "wvT": t3(Wv[sl].T, KT, 128, ES),
            "woT": t3(Wout[:, sl].T, HPC, 128, DIM),
            "se": np.ascontiguousarray(
                (qk_scale[sl] * DIM).reshape(1, ES)).astype(np.float32),
            "onesr": np.ones((1, 128), dtype=np.float32),
            "onec": onec,
            "onecb": onecb,
            "tri": tri,
        })
    return in_maps


def _assemble(results):
    out = np.empty((B, SEQ, DIM), dtype=np.float32)
    for b in range(B):
        acc = results[4 * b]["out"].astype(np.float32)
        for g in range(1, HPC):
            acc = acc + results[4 * b + g]["out"].astype(np.float32)
        out[b] = acc.T
    return out


def kernel(x, Wq, Wk, Wv, Wout, qk_scale):
    nc = _get_program()
    in_maps = _make_in_maps(x, Wq, Wk, Wv, Wout, qk_scale)
    res = run_bass_kernel_spmd(nc, in_maps, core_ids=list(range(NCORES)))
    return _assemble(res.results)
